# revision 47
# baseline (speedup 1.0000x reference)
"""Distributed Trainium2 kernel for rotary causal attention (GPT-NeoX style).

Sharding: tensor-parallel over heads (2 heads per core on 8 cores) for
QKV+rotary+attention; a per-head AllToAll converts head-sharding of z to
sequence-sharding; each core then computes its 256-row slice of the output
projection over ALL heads (no reduction needed); host concatenates slices.

Design notes:
- resid is transposed on the HOST (xT [d, s]); no activation transposes on
  device.
- LayerNormPre centering is folded into the weights on the host
  (W_c = W - col_mean(W), since (x - mean(x)) @ W == x @ W_c), so the device
  only needs rstd = 1/sqrt(E[x^2] + eps). E[x]^2 is O(1/D) of E[x^2] for the
  zero-mean residual stream, far below bf16 noise, so the variance uses
  E[x^2] alone; the column sums of xT^2 come from a ones-matmul and arrive
  broadcast across partitions for free.
- rstd is folded into the rotary tables (rcos = rstd*cos, rsin = rstd*sin)
  for Q/K; rsin commutes into the rotation matmul because the sin table
  repeats per e-pair, so rotary is one matmul + one DVE add per chunk. V
  applies rstd in its ACT-engine PSUM evacuation (Copy with AP scale), via a
  seq-partition rstd column obtained from one PE transpose per tile.
- Attention pairs score tiles two-per-PSUM-bank-pair ([128,1024] f32) so exp
  runs half as many ACT instructions; l/z matmuls and the diagonal exp skip
  fully-masked q-ranges (exact causal triangle); the boundary block is
  masked by a 0/1 triangle multiply on the exp output.
- Softmax is single-pass exp (scores are O(1) after 1/sqrt(d) scale); row
  sums l via ones-matmul; z and l accumulate in fp32 PSUM.
- Phase order maximizes overlap: LN stats run concurrently with head-0
  projections (4+4 PSUM banks); head-0 attention and its AllToAll run while
  head-1 projects (streaming c-outer so each chunk unblocks attention
  early); W_O prefetches as soon as xT frees; the output projection runs in
  two passes (SBUF partial) so head-group-0 matmuls overlap the second
  AllToAll. Output is stored bf16 and upcast on the host.
"""

import os
import sys

import numpy as np

sys.path.insert(0, "/opt/trn_rl_repo")

import ml_dtypes

import concourse.mybir as mybir
import concourse.tile as tile
from concourse import bacc
from concourse.bass import ds
from concourse.bass_utils import run_bass_kernel_spmd

BF16 = mybir.dt.bfloat16
F32 = mybir.dt.float32
AX = mybir.AxisListType
ALU = mybir.AluOpType
ACTF = mybir.ActivationFunctionType

S = 2048          # sequence length
D = 2048          # d_model
NH = 16           # total heads
DH = 128          # head dim
NCORES = 8
HL = NH // NCORES  # heads per core = 2
SLICE = S // NCORES  # output rows per core = 256
ATTN_SCALE = float(np.sqrt(DH))
EPS = 1e-5
NT = S // 128  # 16 seq/d tiles

_cached = {}


def _build_graph(with_qk_bias: bool):
    nc = bacc.Bacc("TRN2", target_bir_lowering=False, debug=False, num_devices=NCORES)

    xT_ext = nc.declare_dram_parameter("xT", [D, S], BF16, isOutput=False)
    wq = nc.declare_dram_parameter("wq", [HL, D, DH], BF16, isOutput=False)
    wk = nc.declare_dram_parameter("wk", [HL, D, DH], BF16, isOutput=False)
    wv = nc.declare_dram_parameter("wv", [HL, D, DH], BF16, isOutput=False)
    wo = nc.declare_dram_parameter("wo", [NH, DH, D], BF16, isOutput=False)
    cosT = nc.declare_dram_parameter("cosT", [DH, S], BF16, isOutput=False)
    sinT = nc.declare_dram_parameter("sinT", [DH, S], BF16, isOutput=False)
    rotT = nc.declare_dram_parameter("rotT", [DH, DH], BF16, isOutput=False)
    tri01 = nc.declare_dram_parameter("tri01", [128, 128], BF16, isOutput=False)
    ident = nc.declare_dram_parameter("ident", [128, 128], BF16, isOutput=False)
    if with_qk_bias:
        bq = nc.declare_dram_parameter("bq", [HL, DH], BF16, isOutput=False)
        bk = nc.declare_dram_parameter("bk", [HL, DH], BF16, isOutput=False)
    out_ext = nc.declare_dram_parameter("out", [SLICE, D], BF16, isOutput=True)

    with tile.TileContext(nc) as tc:
        with (
            tc.tile_pool(name="consts", bufs=1) as consts,
            tc.tile_pool(name="persist", bufs=1) as persist,
            tc.tile_pool(name="dram", bufs=1, space="DRAM") as dram,
        ):
            # ---- constants ----
            cos_sb = consts.tile([128, S], BF16, tag="cos")
            sin_sb = consts.tile([128, S], BF16, tag="sin")
            rot_sb = consts.tile([128, 128], BF16, tag="rot")
            tri_sb = consts.tile([128, 128], BF16, tag="tri")
            ones_sb = consts.tile([128, 128], BF16, tag="ones")
            id_sb = consts.tile([128, 128], BF16, tag="id")
            rcos = consts.tile([128, S], BF16, tag="rcos")
            rsin = consts.tile([128, S], BF16, tag="rsin")
            rstd_bf = consts.tile([128, S], BF16, tag="rstdbf")
            x2acc = consts.tile([128, S], BF16, tag="x2acc")
            v_rstd = consts.tile([128, NT], F32, tag="vrstd")
            eps_sb = consts.tile([128, 1], F32, tag="eps")
            if with_qk_bias:
                bq_sb = consts.tile([1, HL * DH], BF16, tag="bq")
                bk_sb = consts.tile([1, HL * DH], BF16, tag="bk")
                onesrow = consts.tile([1, 512], BF16, tag="onesrow")

            # persistent per-head activations
            q_rot = [persist.tile([128, S], BF16, tag=f"qrot{h}", name=f"qrot{h}") for h in range(HL)]
            k_rot = [persist.tile([128, S], BF16, tag=f"krot{h}", name=f"krot{h}") for h in range(HL)]
            v_nat = [persist.tile([128, S], BF16, tag=f"vnat{h}", name=f"vnat{h}") for h in range(HL)]
            zT = [persist.tile([128, S], BF16, tag=f"zT{h}", name=f"zTh{h}") for h in range(HL)]

            # A2A bounce buffers (DRAM), one pair per head
            a2a_in = [
                dram.tile([NCORES, DH, SLICE], BF16, tag=f"a2a_in{h}", name=f"a2a_in{h}")
                for h in range(HL)
            ]
            a2a_out = [
                dram.tile([NCORES, DH, SLICE], BF16, tag=f"a2a_out{h}", name=f"a2a_out{h}")
                for h in range(HL)
            ]

            with tc.tile_pool(name="xTp", bufs=1) as xTp:
                xT = [xTp.tile([128, S], BF16, tag=f"xT{d}", name=f"xT{d}") for d in range(NT)]
                with tc.tile_pool(name="wqkv", bufs=1) as wqkv:
                    wq_sb = [wqkv.tile([128, NT * DH], BF16, tag=f"wq{h}", name=f"wqs{h}") for h in range(HL)]
                    wk_sb = [wqkv.tile([128, NT * DH], BF16, tag=f"wk{h}", name=f"wks{h}") for h in range(HL)]
                    wv_sb = wqkv.tile([128, NT * HL * DH], BF16, tag="wv")
                    wv_view = wv_sb.rearrange("p (t h e) -> p t h e", t=NT, h=HL)
                    raw_qk = {}
                    with tc.tile_pool(name="rawp", bufs=1) as rawp:
                        for h in range(HL):
                            for tn in ("q", "k"):
                                raw_qk[(h, tn)] = rawp.tile(
                                    [128, S], BF16, tag=f"raw{tn}{h}", name=f"raw{tn}{h}"
                                )

                        # ---- DMA stream: h0 weights, xT interleaved, rest ----
                        for d in range(4):
                            nc.sync.dma_start(xT[d][:], xT_ext[ds(d * 128, 128), :])
                        nc.sync.dma_start(
                            wq_sb[0].rearrange("p (t e) -> p t e", t=NT),
                            wq[0].rearrange("(t p) e -> p t e", p=128),
                        )
                        nc.sync.dma_start(
                            wk_sb[0].rearrange("p (t e) -> p t e", t=NT),
                            wk[0].rearrange("(t p) e -> p t e", p=128),
                        )
                        for d in range(4, NT):
                            nc.sync.dma_start(xT[d][:], xT_ext[ds(d * 128, 128), :])
                        nc.sync.dma_start(cos_sb[:], cosT[:])
                        nc.sync.dma_start(sin_sb[:], sinT[:])
                        nc.sync.dma_start(rot_sb[:], rotT[:])
                        nc.sync.dma_start(tri_sb[:], tri01[:])
                        nc.sync.dma_start(id_sb[:], ident[:])
                        for h in range(HL):
                            nc.sync.dma_start(
                                wv_view[:, :, h, :],
                                wv[h].rearrange("(t p) e -> p t e", p=128),
                            )
                        nc.sync.dma_start(
                            wq_sb[1].rearrange("p (t e) -> p t e", t=NT),
                            wq[1].rearrange("(t p) e -> p t e", p=128),
                        )
                        nc.sync.dma_start(
                            wk_sb[1].rearrange("p (t e) -> p t e", t=NT),
                            wk[1].rearrange("(t p) e -> p t e", p=128),
                        )
                        nc.vector.memset(ones_sb[:], 1.0)
                        nc.vector.memset(eps_sb[:], EPS)
                        if with_qk_bias:
                            for h in range(HL):
                                nc.sync.dma_start(
                                    bq_sb[:, ds(h * DH, DH)], bq[h, :]
                                )
                                nc.sync.dma_start(
                                    bk_sb[:, ds(h * DH, DH)], bk[h, :]
                                )
                            nc.vector.memset(onesrow[:], 1.0)

                        def qk_project(h, tn, w_sb, psP):
                            """Projection matmuls only; raw result to SBUF."""
                            raw = raw_qk[(h, tn)]
                            p_ps = [
                                psP.tile([128, 512], F32, tag=f"proj{c}", name=f"proj{c}")
                                for c in range(4)
                            ]
                            for d in range(NT):
                                for c in range(4):
                                    nc.tensor.matmul(
                                        p_ps[c][:],
                                        w_sb[:, ds(d * DH, DH)],
                                        xT[d][:, ds(c * 512, 512)],
                                        start=(d == 0),
                                        stop=(d == NT - 1) and not with_qk_bias,
                                        skip_group_check=True,
                                    )
                            if with_qk_bias:
                                b_sb = bq_sb if tn == "q" else bk_sb
                                for c in range(4):
                                    nc.tensor.matmul(
                                        p_ps[c][:],
                                        b_sb[:, ds(h * DH, DH)],
                                        onesrow[:],
                                        start=False,
                                        stop=True,
                                        skip_group_check=True,
                                    )
                            for c in range(4):
                                nc.scalar.copy(raw[:, ds(c * 512, 512)], p_ps[c][:])

                        def qk_rotary(h, tn, psR, eng=None):
                            """rot matmul + rotary combine from persisted raw."""
                            if eng is None:
                                eng = nc.gpsimd
                            raw = raw_qk[(h, tn)]
                            tgt = q_rot[h] if tn == "q" else k_rot[h]
                            for c in range(4):
                                s1 = rwork.tile([128, 512], BF16, tag="t1")
                                eng.tensor_tensor(
                                    s1[:], raw[:, ds(c * 512, 512)],
                                    rsin[:, ds(c * 512, 512)], ALU.mult,
                                )
                                t2 = rwork.tile([128, 512], BF16, tag="t2")
                                eng.tensor_tensor(
                                    t2[:], raw[:, ds(c * 512, 512)],
                                    rcos[:, ds(c * 512, 512)], ALU.mult,
                                )
                                r_ps = psR.tile([128, 512], F32, tag="rot")
                                nc.tensor.matmul(
                                    r_ps[:], rot_sb[:], s1[:], start=True, stop=True
                                )
                                nc.vector.tensor_tensor(
                                    tgt[:, ds(c * 512, 512)], r_ps[:], t2[:], ALU.add
                                )

                        def attention(h, psS, psL, psZ):
                            for c in range(4):
                                tmax = 4 * c + 3
                                l_ps = psL.tile([128, 512], F32, tag="l")
                                z_ps = psZ.tile([128, 512], F32, tag="z")
                                for pi in range((tmax + 2) // 2):
                                    t0 = 2 * pi
                                    sT2 = psS.tile([128, 1024], F32, tag="sT2")
                                    pT2 = pTw.tile([128, 1024], BF16, tag="pT2")
                                    for u in range(2):
                                        t = t0 + u
                                        nc.tensor.matmul(
                                            sT2[:, ds(u * 512, 512)],
                                            k_rot[h][:, ds(t * 128, 128)],
                                            q_rot[h][:, ds(c * 512, 512)],
                                            start=True,
                                            stop=True,
                                        )
                                    nc.scalar.activation(
                                        pT2[:], sT2[:], ACTF.Exp, scale=1.0 / ATTN_SCALE
                                    )
                                    for u in range(2):
                                        t = t0 + u
                                        off = max(0, (t - 4 * c) * 128)
                                        if t >= 4 * c:
                                            # boundary block: triangular mask
                                            nc.vector.tensor_tensor(
                                                pT2[:, ds(u * 512 + off, 128)],
                                                pT2[:, ds(u * 512 + off, 128)],
                                                tri_sb[:],
                                                ALU.mult,
                                            )
                                        nc.tensor.matmul(
                                            l_ps[:, ds(off, 512 - off)],
                                            ones_sb[:],
                                            pT2[:, ds(u * 512 + off, 512 - off)],
                                            start=(t == 0),
                                            stop=(t == tmax),
                                            skip_group_check=True,
                                        )
                                        nc.tensor.matmul(
                                            z_ps[:, ds(off, 512 - off)],
                                            v_nat[h][:, ds(t * 128, 128)],
                                            pT2[:, ds(u * 512 + off, 512 - off)],
                                            start=(t == 0),
                                            stop=(t == tmax),
                                            skip_group_check=True,
                                        )
                                rinv = cw.tile([128, 512], F32, tag="rinv")
                                nc.vector.reciprocal(rinv[:], l_ps[:])
                                nc.vector.tensor_tensor(
                                    zT[h][:, ds(c * 512, 512)], z_ps[:], rinv[:], ALU.mult
                                )
                                # stream this chunk's A2A staging DMAs now
                                for j in (2 * c, 2 * c + 1):
                                    nc.sync.dma_start(
                                        a2a_in[h][j, :, :],
                                        zT[h][:, ds(j * SLICE, SLICE)],
                                    )
                            nc.gpsimd.collective_compute(
                                "AllToAll",
                                ALU.bypass,
                                ins=[a2a_in[h].opt()],
                                outs=[a2a_out[h].opt()],
                                replica_groups=[list(range(NCORES))],
                            )

                        with (
                            tc.tile_pool(name="rwork", bufs=4) as rwork,
                        ):
                            # -- LN stats (DVE chain) + interleaved h0 Q/K
                            # projections (8 banks, both ride the xT stream) --
                            with (
                                tc.tile_pool(name="sqp", bufs=3) as sqp,
                                tc.tile_pool(name="statw", bufs=2) as statw,
                            ):
                                # Var(x) ~= E[x^2]: E[x]^2 is O(1/D) of E[x^2]
                                # for the (near-zero-mean) residual stream, far
                                # below bf16 noise; centering itself is exact
                                # via the host-centered weights. Cross-tile
                                # d-reduction is elementwise on DVE.
                                nc.vector.tensor_tensor(
                                    x2acc[:], xT[0][:], xT[0][:], ALU.mult
                                )
                                for d in range(1, NT):
                                    x2 = sqp.tile([128, S], BF16, tag="x2")
                                    nc.vector.tensor_tensor(
                                        x2[:], xT[d][:], xT[d][:], ALU.mult
                                    )
                                    nc.vector.tensor_tensor(
                                        x2acc[:], x2acc[:], x2[:], ALU.add
                                    )

                                with tc.tile_pool(name="psP8", bufs=1, space="PSUM") as psP8:
                                    q_ps = [
                                        psP8.tile([128, 512], F32, tag=f"pq{c}", name=f"pq{c}")
                                        for c in range(4)
                                    ]
                                    k_ps = [
                                        psP8.tile([128, 512], F32, tag=f"pk{c}", name=f"pk{c}")
                                        for c in range(4)
                                    ]
                                    for d in range(NT):
                                        for c in range(4):
                                            nc.tensor.matmul(
                                                q_ps[c][:],
                                                wq_sb[0][:, ds(d * DH, DH)],
                                                xT[d][:, ds(c * 512, 512)],
                                                start=(d == 0),
                                                stop=(d == NT - 1) and not with_qk_bias,
                                                skip_group_check=True,
                                            )
                                            nc.tensor.matmul(
                                                k_ps[c][:],
                                                wk_sb[0][:, ds(d * DH, DH)],
                                                xT[d][:, ds(c * 512, 512)],
                                                start=(d == 0),
                                                stop=(d == NT - 1) and not with_qk_bias,
                                                skip_group_check=True,
                                            )
                                    if with_qk_bias:
                                        for c in range(4):
                                            nc.tensor.matmul(
                                                q_ps[c][:],
                                                bq_sb[:, ds(0, DH)],
                                                onesrow[:],
                                                start=False,
                                                stop=True,
                                                skip_group_check=True,
                                            )
                                            nc.tensor.matmul(
                                                k_ps[c][:],
                                                bk_sb[:, ds(0, DH)],
                                                onesrow[:],
                                                start=False,
                                                stop=True,
                                                skip_group_check=True,
                                            )
                                    for c in range(4):
                                        nc.scalar.copy(
                                            raw_qk[(0, "q")][:, ds(c * 512, 512)],
                                            q_ps[c][:],
                                        )
                                        nc.scalar.copy(
                                            raw_qk[(0, "k")][:, ds(c * 512, 512)],
                                            k_ps[c][:],
                                        )

                                with tc.tile_pool(name="psStat", bufs=1, space="PSUM") as psStat:
                                    s2_ps = [
                                        psStat.tile([128, 512], F32, tag=f"sq{c}", name=f"sq{c}")
                                        for c in range(4)
                                    ]
                                    for c in range(4):
                                        nc.tensor.matmul(
                                            s2_ps[c][:],
                                            ones_sb[:],
                                            x2acc[:, ds(c * 512, 512)],
                                            start=True,
                                            stop=True,
                                        )
                                        # rstd = 1/sqrt(E[x^2] + eps), broadcast
                                        std = statw.tile([128, 512], F32, tag="std")
                                        nc.scalar.activation(
                                            std[:], s2_ps[c][:], ACTF.Sqrt,
                                            scale=1.0 / D, bias=eps_sb[:],
                                        )
                                        with nc.allow_low_precision(
                                            reason="rstd to bf16; 0.4% LN-scale noise is within budget"
                                        ):
                                            nc.vector.reciprocal(
                                                rstd_bf[:, ds(c * 512, 512)], std[:]
                                            )
                                    # pre-warm the Exp table so attention's
                                    # first exp skips the 1.3us table switch
                                    warm = statw.tile([1, 1], F32, tag="warm")
                                    nc.scalar.activation(
                                        warm[:], eps_sb[0:1, 0:1], ACTF.Exp
                                    )

                            # rstd folded into rotary tables for Q/K
                            nc.vector.tensor_tensor(rcos[:], cos_sb[:], rstd_bf[:], ALU.mult)
                            nc.vector.tensor_tensor(rsin[:], sin_sb[:], rstd_bf[:], ALU.mult)

                            # -- rotary h0, v_rstd, V projection --
                            with (
                                tc.tile_pool(name="psR1", bufs=2, space="PSUM") as psR1,
                                tc.tile_pool(name="psV", bufs=3, space="PSUM") as psV,
                            ):
                                # v_rstd[:, j]: rstd in seq-partition layout via
                                # PE transpose of a row-broadcast block
                                for j in range(NT):
                                    tr = psR1.tile([128, 512], BF16, tag="rot")
                                    nc.tensor.transpose(
                                        tr[:, 0:128], rstd_bf[:, ds(j * 128, 128)], id_sb[:]
                                    )
                                    nc.vector.tensor_copy(v_rstd[:, j : j + 1], tr[:, 0:1])
                                qk_rotary(0, "q", psR1)
                                qk_rotary(0, "k", psR1)
                                # V: natural layout [s(128), h*e]; rstd applied
                                # in the ACT evacuation (Copy with AP scale)
                                for j in range(NT):
                                    v_ps = psV.tile([128, HL * DH], F32, tag="vproj")
                                    for d in range(NT):
                                        nc.tensor.matmul(
                                            v_ps[:],
                                            xT[d][:, ds(j * 128, 128)],
                                            wv_view[:, d, :, :].rearrange("p h e -> p (h e)"),
                                            start=(d == 0),
                                            stop=(d == NT - 1),
                                        )
                                    for h in range(HL):
                                        nc.scalar.activation(
                                            v_nat[h][:, ds(j * 128, 128)],
                                            v_ps[:, ds(h * DH, DH)],
                                            ACTF.Copy,
                                            scale=v_rstd[:, j : j + 1],
                                        )

                            with (
                                tc.tile_pool(name="pTw", bufs=4) as pTw,
                                tc.tile_pool(name="cw", bufs=3) as cw,
                            ):
                                def attention(h, psS, psL, psZ):
                                    for c in range(4):
                                        tmax = 4 * c + 3
                                        l_ps = psL.tile([128, 512], F32, tag="l")
                                        z_ps = psZ.tile([128, 512], F32, tag="z")
                                        for pi in range((tmax + 2) // 2):
                                            t0 = 2 * pi
                                            sT2 = psS.tile([128, 1024], F32, tag="sT2")
                                            pT2 = pTw.tile([128, 1024], BF16, tag="pT2")
                                            for u in range(2):
                                                t = t0 + u
                                                nc.tensor.matmul(
                                                    sT2[:, ds(u * 512, 512)],
                                                    k_rot[h][:, ds(t * 128, 128)],
                                                    q_rot[h][:, ds(c * 512, 512)],
                                                    start=True,
                                                    stop=True,
                                                )
                                            e0 = max(0, (t0 - 4 * c) * 128)
                                            nc.scalar.activation(
                                                pT2[:, ds(e0, 1024 - e0)],
                                                sT2[:, ds(e0, 1024 - e0)],
                                                ACTF.Exp,
                                                scale=1.0 / ATTN_SCALE,
                                            )
                                            for u in range(2):
                                                t = t0 + u
                                                off = max(0, (t - 4 * c) * 128)
                                                if t >= 4 * c:
                                                    # boundary: triangular mask
                                                    nc.vector.tensor_tensor(
                                                        pT2[:, ds(u * 512 + off, 128)],
                                                        pT2[:, ds(u * 512 + off, 128)],
                                                        tri_sb[:],
                                                        ALU.mult,
                                                    )
                                                nc.tensor.matmul(
                                                    l_ps[:, ds(off, 512 - off)],
                                                    ones_sb[:],
                                                    pT2[:, ds(u * 512 + off, 512 - off)],
                                                    start=(t == 0),
                                                    stop=(t == tmax),
                                                    skip_group_check=True,
                                                )
                                                nc.tensor.matmul(
                                                    z_ps[:, ds(off, 512 - off)],
                                                    v_nat[h][:, ds(t * 128, 128)],
                                                    pT2[:, ds(u * 512 + off, 512 - off)],
                                                    start=(t == 0),
                                                    stop=(t == tmax),
                                                    skip_group_check=True,
                                                )
                                        rinv = cw.tile([128, 512], F32, tag="rinv")
                                        nc.vector.reciprocal(rinv[:], l_ps[:])
                                        nc.vector.tensor_tensor(
                                            zT[h][:, ds(c * 512, 512)],
                                            z_ps[:], rinv[:], ALU.mult,
                                        )
                                        # stream this chunk's A2A staging DMAs
                                        for j in (2 * c, 2 * c + 1):
                                            nc.sync.dma_start(
                                                a2a_in[h][j, :, :],
                                                zT[h][:, ds(j * SLICE, SLICE)],
                                            )
                                    nc.gpsimd.collective_compute(
                                        "AllToAll",
                                        ALU.bypass,
                                        ins=[a2a_in[h].opt()],
                                        outs=[a2a_out[h].opt()],
                                        replica_groups=[list(range(NCORES))],
                                    )

                                # -- head-0 attention + its A2A (early) --
                                with (
                                    tc.tile_pool(name="psS0", bufs=3, space="PSUM") as psS0,
                                    tc.tile_pool(name="psL0", bufs=1, space="PSUM") as psL0,
                                    tc.tile_pool(name="psZ0", bufs=1, space="PSUM") as psZ0,
                                ):
                                    attention(0, psS0, psL0, psZ0)

                                # -- head-1 projections + rotary (last xT use) --
                                with (
                                    tc.tile_pool(name="psP2", bufs=2, space="PSUM") as psP2,
                                    tc.tile_pool(name="psR2", bufs=2, space="PSUM") as psR2,
                                ):
                                    # c-outer for head 1: each chunk's rotary
                                    # (and attention tile) unblocks early
                                    for tn, w_sb in (("k", wk_sb[1]), ("q", wq_sb[1])):
                                        raw = raw_qk[(1, tn)]
                                        tgt = q_rot[1] if tn == "q" else k_rot[1]
                                        for c in range(4):
                                            p_ps = psP2.tile([128, 512], F32, tag="proj")
                                            for d in range(NT):
                                                nc.tensor.matmul(
                                                    p_ps[:],
                                                    w_sb[:, ds(d * DH, DH)],
                                                    xT[d][:, ds(c * 512, 512)],
                                                    start=(d == 0),
                                                    stop=(d == NT - 1) and not with_qk_bias,
                                                )
                                            if with_qk_bias:
                                                b_sb = bq_sb if tn == "q" else bk_sb
                                                nc.tensor.matmul(
                                                    p_ps[:],
                                                    b_sb[:, ds(1 * DH, DH)],
                                                    onesrow[:],
                                                    start=False,
                                                    stop=True,
                                                )
                                            nc.scalar.copy(
                                                raw[:, ds(c * 512, 512)], p_ps[:]
                                            )
                                            s1 = rwork.tile([128, 512], BF16, tag="t1")
                                            nc.vector.tensor_tensor(
                                                s1[:], raw[:, ds(c * 512, 512)],
                                                rsin[:, ds(c * 512, 512)], ALU.mult,
                                            )
                                            t2 = rwork.tile([128, 512], BF16, tag="t2")
                                            nc.vector.tensor_tensor(
                                                t2[:], raw[:, ds(c * 512, 512)],
                                                rcos[:, ds(c * 512, 512)], ALU.mult,
                                            )
                                            r_ps = psR2.tile([128, 512], F32, tag="rot")
                                            nc.tensor.matmul(
                                                r_ps[:], rot_sb[:], s1[:],
                                                start=True, stop=True,
                                            )
                                            nc.vector.tensor_tensor(
                                                tgt[:, ds(c * 512, 512)],
                                                r_ps[:], t2[:], ALU.add,
                                            )

            # xT/weights/raw freed; W_O prefetch + head-1 attention + output
            with (
                tc.tile_pool(name="wop", bufs=1) as wop,
                tc.tile_pool(name="pTw1", bufs=4) as pTw,
                tc.tile_pool(name="cw1", bufs=4) as cw,
            ):
                wo_sb = [wop.tile([128, D], BF16, tag=f"wo{g}", name=f"wos{g}") for g in range(NH)]
                # head-group-0 weights first: Wo-hg0 starts during A2A#2
                for g in list(range(0, NH, HL)) + list(range(1, NH, HL)):
                    nc.sync.dma_start(wo_sb[g][:], wo[g, :, :])

                with (
                    tc.tile_pool(name="psS1", bufs=3, space="PSUM") as psS1,
                    tc.tile_pool(name="psL1", bufs=1, space="PSUM") as psL1,
                    tc.tile_pool(name="psZ1", bufs=1, space="PSUM") as psZ1,
                ):
                    def attention1(h):
                        for c in range(4):
                            tmax = 4 * c + 3
                            l_ps = psL1.tile([128, 512], F32, tag="l")
                            z_ps = psZ1.tile([128, 512], F32, tag="z")
                            for pi in range((tmax + 2) // 2):
                                t0 = 2 * pi
                                sT2 = psS1.tile([128, 1024], F32, tag="sT2")
                                pT2 = pTw.tile([128, 1024], BF16, tag="pT2")
                                for u in range(2):
                                    t = t0 + u
                                    nc.tensor.matmul(
                                        sT2[:, ds(u * 512, 512)],
                                        k_rot[h][:, ds(t * 128, 128)],
                                        q_rot[h][:, ds(c * 512, 512)],
                                        start=True,
                                        stop=True,
                                    )
                                e0 = max(0, (t0 - 4 * c) * 128)
                                nc.scalar.activation(
                                    pT2[:, ds(e0, 1024 - e0)],
                                    sT2[:, ds(e0, 1024 - e0)],
                                    ACTF.Exp,
                                    scale=1.0 / ATTN_SCALE,
                                )
                                for u in range(2):
                                    t = t0 + u
                                    off = max(0, (t - 4 * c) * 128)
                                    if t >= 4 * c:
                                        nc.vector.tensor_tensor(
                                            pT2[:, ds(u * 512 + off, 128)],
                                            pT2[:, ds(u * 512 + off, 128)],
                                            tri_sb[:],
                                            ALU.mult,
                                        )
                                    nc.tensor.matmul(
                                        l_ps[:, ds(off, 512 - off)],
                                        ones_sb[:],
                                        pT2[:, ds(u * 512 + off, 512 - off)],
                                        start=(t == 0),
                                        stop=(t == tmax),
                                        skip_group_check=True,
                                    )
                                    nc.tensor.matmul(
                                        z_ps[:, ds(off, 512 - off)],
                                        v_nat[h][:, ds(t * 128, 128)],
                                        pT2[:, ds(u * 512 + off, 512 - off)],
                                        start=(t == 0),
                                        stop=(t == tmax),
                                        skip_group_check=True,
                                    )
                            rinv = cw.tile([128, 512], F32, tag="rinv")
                            nc.vector.reciprocal(rinv[:], l_ps[:])
                            nc.vector.tensor_tensor(
                                zT[h][:, ds(c * 512, 512)], z_ps[:], rinv[:], ALU.mult
                            )
                            for j in (2 * c, 2 * c + 1):
                                nc.sync.dma_start(
                                    a2a_in[h][j, :, :],
                                    zT[h][:, ds(j * SLICE, SLICE)],
                                )
                        nc.gpsimd.collective_compute(
                            "AllToAll",
                            ALU.bypass,
                            ins=[a2a_in[h].opt()],
                            outs=[a2a_out[h].opt()],
                            replica_groups=[list(range(NCORES))],
                        )

                    attention1(1)

                # ========== Phase D: output projection, two psum passes ==========
                with (
                    tc.tile_pool(name="zap", bufs=1) as zap,
                    tc.tile_pool(name="psO", bufs=1, space="PSUM") as psO,
                    tc.tile_pool(name="prt", bufs=1) as prt,
                    tc.tile_pool(name="ostg", bufs=2) as ostg,
                ):
                    zTa = [zap.tile([128, SLICE], BF16, tag=f"zTa{g}", name=f"zTa{g}") for g in range(NH)]
                    partial = [
                        prt.tile([128, 512], BF16, tag=f"prt{i}", name=f"prt{i}")
                        for i in range(8)
                    ]
                    for hg in range(HL):
                        for g in range(hg, NH, HL):
                            # head g lives on core g//HL, local index g%HL
                            nc.sync.dma_start(zTa[g][:], a2a_out[hg][g // HL, :, :])
                        for sr in range(SLICE // 128):
                            for c in range(4):
                                o_ps = psO.tile([128, 512], F32, tag=f"o{sr * 4 + c}")
                                if hg == 1:
                                    # fold the group-0 partial back in via an
                                    # identity matmul (PE) instead of a DVE
                                    # add on the critical tail
                                    nc.tensor.matmul(
                                        o_ps[:],
                                        id_sb[:],
                                        partial[sr * 4 + c][:],
                                        start=True,
                                        stop=False,
                                        skip_group_check=True,
                                    )
                                for gi, g in enumerate(range(hg, NH, HL)):
                                    nc.tensor.matmul(
                                        o_ps[:],
                                        zTa[g][:, ds(sr * 128, 128)],
                                        wo_sb[g][:, ds(c * 512, 512)],
                                        start=(hg == 0 and gi == 0) or (hg == 1 and False),
                                        stop=(gi == NCORES - 1),
                                        skip_group_check=True,
                                    )
                                if hg == 0:
                                    nc.vector.tensor_copy(
                                        partial[sr * 4 + c][:], o_ps[:]
                                    )
                                else:
                                    stg = ostg.tile([128, 512], BF16, tag="stg")
                                    nc.scalar.copy(stg[:], o_ps[:])
                                    nc.sync.dma_start(
                                        out_ext[ds(sr * 128, 128), ds(c * 512, 512)],
                                        stg[:],
                                    )
    nc.compile()
    return nc


def _rotary_tables():
    pos = np.arange(S, dtype=np.float64)
    dim = np.arange(DH // 2, dtype=np.float64)
    freq = 10000.0 ** (dim / (DH / 2))
    freq = np.repeat(freq, 2)  # interleaved
    ang = pos[:, None] / freq[None, :]  # [S, DH]
    return np.sin(ang).T.copy(), np.cos(ang).T.copy()  # [DH, S]


def build_in_maps(inputs):
    """Build the per-core input maps from the full input dict."""
    resid_pre = np.asarray(inputs["resid_pre"], np.float32)
    W_Q = np.asarray(inputs["W_Q"], np.float32)
    W_K = np.asarray(inputs["W_K"], np.float32)
    W_V = np.asarray(inputs["W_V"], np.float32)
    W_O = np.asarray(inputs["W_O"], np.float32)
    b_Q = np.asarray(inputs["b_Q"], np.float32)
    b_K = np.asarray(inputs["b_K"], np.float32)

    bf = ml_dtypes.bfloat16
    sinT, cosT = _rotary_tables()
    rotT = np.zeros((DH, DH), np.float32)
    idx = np.arange(0, DH, 2)
    rotT[idx, idx + 1] = 1.0   # rotT = R^T with R[2i,2i+1]=-1, R[2i+1,2i]=1
    rotT[idx + 1, idx] = -1.0
    tri01 = (np.arange(128)[:, None] <= np.arange(128)[None, :]).astype(np.float32)

    with_qk_bias = bool(np.any(b_Q) or np.any(b_K))

    # center the d-axis of the projection weights: (x-mean(x))@W == x@Wc
    Wq_c = W_Q - W_Q.mean(axis=1, keepdims=True)
    Wk_c = W_K - W_K.mean(axis=1, keepdims=True)
    Wv_c = W_V - W_V.mean(axis=1, keepdims=True)

    xT = np.ascontiguousarray(resid_pre[0].T).astype(bf)
    common = dict(
        xT=xT,
        wo=W_O.astype(bf),
        cosT=cosT.astype(bf),
        sinT=sinT.astype(bf),
        rotT=rotT.astype(bf),
        tri01=tri01.astype(bf),
        ident=np.eye(128, dtype=bf),
    )
    in_maps = []
    for i in range(NCORES):
        hs = slice(HL * i, HL * (i + 1))
        m = dict(common)
        m["wq"] = Wq_c[hs].astype(bf)
        m["wk"] = Wk_c[hs].astype(bf)
        m["wv"] = Wv_c[hs].astype(bf)
        if with_qk_bias:
            m["bq"] = np.ascontiguousarray(b_Q[hs]).astype(bf)
            m["bk"] = np.ascontiguousarray(b_K[hs]).astype(bf)
        in_maps.append(m)
    return in_maps


def kernel(resid_pre, W_Q, W_K, W_V, W_O, b_Q, b_K, b_V, b_O):
    inputs = dict(
        resid_pre=resid_pre, W_Q=W_Q, W_K=W_K, W_V=W_V, W_O=W_O,
        b_Q=b_Q, b_K=b_K, b_V=b_V, b_O=b_O,
    )
    in_maps = build_in_maps(inputs)
    with_qk_bias = "bq" in in_maps[0]

    key = ("nc", with_qk_bias)
    if key not in _cached:
        _cached[key] = _build_graph(with_qk_bias)
    nc = _cached[key]

    trace = bool(int(os.environ.get("KTRACE", "0")))
    try:
        res = run_bass_kernel_spmd(nc, in_maps, list(range(NCORES)), trace=trace)
    except ModuleNotFoundError:
        # NTFF profiling hooks unavailable in this environment
        res = run_bass_kernel_spmd(nc, in_maps, list(range(NCORES)), trace=False)
    _cached["last_result"] = res

    out = np.concatenate(
        [np.asarray(res.results[i]["out"], np.float32) for i in range(NCORES)], axis=0
    )
    # exact host-side bias fold: z = attn@v + b_V (softmax rows sum to 1)
    b_V64 = np.asarray(b_V, np.float64)
    corr = np.einsum("he,hed->d", b_V64, np.asarray(W_O, np.float64))
    corr = (corr + np.asarray(b_O, np.float64)).astype(np.float32)
    return (out + corr[None, :])[None]
